# revision 8
# baseline (speedup 1.0000x reference)
"""Causal self-attention (B=2, T=2048, C=1024, H=16) on 8 trn2 NeuronCores.

Sharding: Megatron-style tensor parallel crossed with data parallel.
Core cid = 4*b + g handles batch b (of 2) and head group g (4 heads of 16).
Each core computes its 4 heads' attention plus the partial output
projection (w_proj rows for those heads); the host sums the 4 partials
per batch and adds b_proj. No device collectives needed.

Matmul operands are bf16 (inputs rounded on host), accumulation fp32 in
PSUM; softmax runs in fp32 (exp reads PSUM directly). Everything stays
in "transposed space" so no on-device transposes are needed: the host
passes x[b].T:
  - q^T/k^T come from  lhsT=w_qk[c,j],  rhs=xT[c,t]
  - V      comes from  lhsT=xT[c,t],    rhs=w_v[c,j]
  - S^T    comes from  lhsT=k^T[d,tk],  rhs=q^T[d,tq]   (K=64, head pairs
                                                          stacked on partitions)
  - y^T    comes from  lhsT=V[tk,d|1],  rhs=P^T[tk,tq]  (ones col -> l)
  - out    comes from  lhsT=y^T[d,t],   rhs=w_proj[d,c]
Softmax skips max-subtraction (logits ~N(0,1), |s|<~7, exp safe in fp32);
causal masking multiplies only the 128-wide triangle of each diagonal
block by a precomputed [128,128] staircase after the exp.

Scheduling: the attention j-loops are the backbone of the PE stream;
everything else is stuffed into them as filler via two FIFOs popped one
step per S/AV slot:
  - urgent: next chunk's QKV matmul half-groups (must finish before that
    chunk's attention starts; force-drained at the chunk boundary)
  - lazy:   softmax-normalize steps (col-quartered DVE reciprocals, K=1
    ones-broadcast matmuls of 1/l, per-pair yT multiplies) and the
    previous chunk's projection steps
The denominator row l (harvested from a ones column in V) lands on PSUM
partition 64 of each head's AV accumulator; the Pool engine copies it to
l4 partition 32h (chunk-slotted so lazy normalize steps can lag), and
Pool also does all proj PSUM->SBUF copies, keeping DVE off the critical
exp->mask->AV path.
"""

import numpy as np

B, T, C, H = 2, 2048, 1024, 16
HD = C // H  # 64
P = 128
NKT = C // P  # 8 k-tiles over the embedding dim
TCH = 512  # t-chunk (q) width
NCH = T // TCH  # 4 q-chunks
NTB = T // P  # 16 t-blocks (k) per sequence
HPC = 4  # heads per core
DC = HPC * HD  # 256 head dims per core

_CACHE = {}


def _build():
    import concourse.mybir as mybir
    from concourse import bacc
    from concourse.tile import TileContext

    F32 = mybir.dt.float32
    F32R = mybir.dt.float32r
    BF16 = mybir.dt.bfloat16
    AF = mybir.ActivationFunctionType

    nc = bacc.Bacc("TRN2", target_bir_lowering=False, debug=False)

    xT = nc.dram_tensor("xT", (C, T), BF16, kind="ExternalInput")
    wqk = nc.dram_tensor("wqk", (C, 2 * DC), BF16, kind="ExternalInput")
    wv = nc.dram_tensor("wv", (C, DC), BF16, kind="ExternalInput")
    wproj = nc.dram_tensor("wproj", (DC, C), BF16, kind="ExternalInput")
    bqk = nc.dram_tensor("bqk", (P, 4), F32, kind="ExternalInput")
    # bv extended with a ones column at [..., HD] (feeds V's l-sum column)
    bv = nc.dram_tensor("bv", (1, HPC, HD + 1), F32, kind="ExternalInput")
    # [128,128] staircase: keep if partition p <= column i
    mask_tri = nc.dram_tensor("mask_tri", (P, P), BF16, kind="ExternalInput")
    emat = nc.dram_tensor("emat", (4, 2, P), F32R, kind="ExternalInput")
    out = nc.dram_tensor("out", (T, C), F32, kind="ExternalOutput")

    with TileContext(nc) as tc:
        with (
            tc.tile_pool(name="persist", bufs=1) as pp,
            tc.tile_pool(name="consts", bufs=1) as cp,
        ):
            # ---- persistent SBUF ----
            wqk_sb = pp.tile([P, NKT, 2 * DC], BF16)  # 8KB/part
            wv_sb = pp.tile([P, NKT, DC], BF16)  # 4KB
            wproj_sb = pp.tile([P, DC // P, C], BF16)  # 4KB
            bqk_sb = cp.tile([P, 4], F32)
            bv_sb = cp.tile([P, HPC, HD + 1], F32)
            mask_sb = cp.tile([P, P], BF16)
            # E-matrix lhsT for the 1/l broadcast: K=4 rows pick head
            # 2p (cols 0:64) and 2p+1 (cols 64:128) for pair p
            emat_sb = cp.tile([4, 2, P], F32R)
            # head-PAIR q^T/k^T tiles: pair p holds head 2p on partitions
            # 0:64 and head 2p+1 on 64:128 (S matmuls run K=64 at base 0/64)
            qts = [
                pp.tile([P, T], BF16, tag=f"qt{p}", name=f"qt{p}")
                for p in range(2)
            ]
            kts = [
                pp.tile([P, T], BF16, tag=f"kt{p}", name=f"kt{p}")
                for p in range(2)
            ]
            v_sb = pp.tile([P, NTB, HPC, HD + 1], BF16)  # 8.1KB (+ones col)
            yT_sb = pp.tile([P, DC // P, T], BF16)  # 8KB
            # denominator rows: head h -> partition h (written by DMA
            # straight from PSUM, which has no partition-base limits),
            # one chunk slot each so lazily-scheduled normalizes never
            # read a slot the next chunk has already overwritten.
            l4_sb = cp.tile([4, NCH, TCH], F32)
            rec4_sb = cp.tile([4, TCH], F32R)

            xT_r = xT[:].rearrange("(kt p) t -> p kt t", p=P)
            wqk_r = wqk[:].rearrange("(kt p) j -> p kt j", p=P)
            wv_r = wv[:].rearrange("(kt p) j -> p kt j", p=P)
            wproj_r = wproj[:].rearrange("(kt p) n -> p kt n", p=P)

            with (
                tc.tile_pool(name="xin", bufs=4) as xpool,
                tc.tile_pool(name="ps_s", bufs=4, space="PSUM") as ps_s,
                tc.tile_pool(name="ps_y", bufs=2, space="PSUM") as ps_y,
                tc.tile_pool(name="ps_o", bufs=2, space="PSUM") as ps_o,
                tc.tile_pool(name="pt", bufs=8) as ptp,
                tc.tile_pool(name="outs", bufs=3) as outp,
                tc.tile_pool(name="lst", bufs=2) as lstp,
            ):
                # ---- prologue DMAs, completion-priority order ----
                x_tiles = [
                    xpool.tile([P, NKT, TCH], BF16, tag="x_tile", name=f"x{a}")
                    for a in range(NCH)
                ]
                # x0/wqk interleaved per-kt so the first qk1 matmuls can
                # start after ~1 kt instead of after the whole tensor
                for kt in range(NKT):
                    nc.sync.dma_start(
                        x_tiles[0][:, kt, :], xT_r[:, kt, 0:TCH]
                    )
                    nc.sync.dma_start(wqk_sb[:, kt, :], wqk_r[:, kt, :])
                nc.sync.dma_start(bqk_sb[:], bqk[:])
                nc.sync.dma_start(bv_sb[:], bv[:].to_broadcast((P, HPC, HD + 1)))
                nc.sync.dma_start(mask_sb[:], mask_tri[:])
                nc.sync.dma_start(emat_sb[:], emat[:])
                for kt in range(NKT):
                    nc.sync.dma_start(wv_sb[:, kt, :], wv_r[:, kt, :])
                for a in range(1, NCH):
                    ch = slice(a * TCH, (a + 1) * TCH)
                    nc.sync.dma_start(x_tiles[a][:, 0:4, :], xT_r[:, 0:4, ch])
                    nc.sync.dma_start(
                        x_tiles[a][:, 4:NKT, :], xT_r[:, 4:NKT, ch]
                    )
                for kt in range(DC // P):
                    nc.sync.dma_start(wproj_sb[:, kt, :], wproj_r[:, kt, :])

                # ---- QKV steps (emitted as half-groups of 4 matmuls) ----
                def make_qkv_steps(a):
                    """Return the 16 half-group steps for chunk a."""
                    ch = slice(a * TCH, (a + 1) * TCH)
                    x_tile = x_tiles[a]
                    steps = []
                    state = {}

                    def qk1(jt, half):
                        if half == 0:
                            state[jt] = ps_s.tile(
                                [P, TCH], F32, tag="ps", name=f"pq{a}_{jt}"
                            )
                        pq = state[jt]
                        for kt in range(4 * half, 4 * half + 4):
                            nc.tensor.matmul(
                                pq[:],
                                wqk_sb[:, kt, jt * P : (jt + 1) * P],
                                x_tile[:, kt, :],
                                start=(kt == 0),
                                stop=(kt == NKT - 1),
                            )
                        if half == 1:
                            # out = in*scale + bias; q carries the 1/sqrt(hd)
                            # scale (host pre-scaled the q bias); two
                            # half-width ops keep the DVE stream fine-grained
                            dst = qts[jt] if jt < 2 else kts[jt - 2]
                            sc = 0.125 if jt < 2 else 1.0
                            for cx in range(2):
                                csl = slice(
                                    a * TCH + cx * 256, a * TCH + cx * 256 + 256
                                )
                                nc.vector.tensor_scalar(
                                    dst[:, csl],
                                    pq[:, cx * 256 : cx * 256 + 256],
                                    sc,
                                    bqk_sb[:, jt : jt + 1],
                                    mybir.AluOpType.mult,
                                    mybir.AluOpType.add,
                                )
                            del state[jt]

                    def v1(tb, half):
                        key = ("v", tb)
                        if half == 0:
                            state[key] = ps_s.tile(
                                [P, HPC, HD], F32, tag="ps", name=f"pv{a}_{tb}"
                            )
                        pv = state[key]
                        for kt in range(4 * half, 4 * half + 4):
                            nc.tensor.matmul(
                                pv[:],
                                x_tile[:, kt, tb * P : (tb + 1) * P],
                                wv_sb[:, kt, :],
                                start=(kt == 0),
                                stop=(kt == NKT - 1),
                            )
                        if half == 1:
                            tg = a * (TCH // P) + tb
                            nc.vector.tensor_add(
                                v_sb[:, tg, :, 0:HD], pv[:], bv_sb[:, :, 0:HD]
                            )
                            # ones column for the softmax-denominator row
                            nc.vector.tensor_copy(
                                v_sb[:, tg, :, HD : HD + 1],
                                bv_sb[:, :, HD : HD + 1],
                            )
                            del state[key]

                    for jt in range(4):
                        for half in range(2):
                            steps.append(lambda jt=jt, h=half: qk1(jt, h))
                    for tb in range(4):
                        for half in range(2):
                            steps.append(lambda tb=tb, h=half: v1(tb, h))
                    return steps

                def proj_steps(a):
                    """Projection of chunk a: 8 steps of [2 MMs + copy + DMA]."""
                    steps = []

                    def pstep(tb, ncx):
                        tg = a * (TCH // P) + tb
                        po = ps_o.tile(
                            [P, TCH], F32, tag="po", name=f"po{a}_{tb}_{ncx}"
                        )
                        for kt in range(DC // P):
                            nc.tensor.matmul(
                                po[:],
                                yT_sb[:, kt, tg * P : (tg + 1) * P],
                                wproj_sb[:, kt, ncx * TCH : (ncx + 1) * TCH],
                                start=(kt == 0),
                                stop=(kt == DC // P - 1),
                            )
                        o_tile = outp.tile([P, TCH], F32, tag="osb")
                        nc.vector.tensor_copy(o_tile[:], po[:])
                        nc.sync.dma_start(
                            out[tg * P : (tg + 1) * P, ncx * TCH : (ncx + 1) * TCH],
                            o_tile[:],
                        )

                    for tb in range(TCH // P):
                        for ncx in range(2):
                            steps.append(lambda tb=tb, ncx=ncx: pstep(tb, ncx))
                    return steps

                def norm_steps(a):
                    """Normalize steps for chunk a: 4 col-quarter
                    reciprocals over all 4 heads' l rows (fine-grained so
                    the in-order DVE stays responsive), then per pair a
                    K=4 E-matrix broadcast matmul of 1/l and an in-place
                    yT multiply."""
                    steps = []
                    ch = slice(a * TCH, (a + 1) * TCH)

                    def recq(q):
                        cs = slice(q * P, q * P + P)
                        with nc.allow_low_precision(
                            reason="f32r recip feeds broadcast matmul; l>=1"
                        ):
                            nc.vector.reciprocal(
                                rec4_sb[:, cs], l4_sb[:, a, cs]
                            )

                    state = {}

                    def rbmm(pair):
                        rb = ps_o.tile(
                            [P, TCH], F32, tag="po", name=f"rb{a}_{pair}"
                        )
                        state["rb"] = rb
                        nc.tensor.matmul(
                            rb[:],
                            emat_sb[:, pair, :],
                            rec4_sb[:],
                            start=True,
                            stop=True,
                        )

                    def ymul(pair):
                        ysl = yT_sb[:, pair, ch]
                        nc.vector.tensor_mul(ysl, ysl, state["rb"][:])
                        del state["rb"]

                    for q in range(4):
                        steps.append(lambda q=q: recq(q))
                    for pair in range(2):
                        steps.append(lambda pair=pair: rbmm(pair))
                        steps.append(lambda pair=pair: ymul(pair))
                    return steps

                urgent = []  # next chunk's QKV: must land before its attention
                lazy = []  # normalize + projection: whenever there's a slot

                def pop_carry():
                    if urgent:
                        urgent.pop(0)()
                    elif lazy:
                        lazy.pop(0)()

                # chunk 0 QKV inline, ordered so heads 0/1 unblock first
                steps0 = make_qkv_steps(0)
                for i in (0, 1, 4, 5):  # qk1 jt=0 (q pair0), jt=2 (k pair0)
                    steps0[i]()
                for i in range(8, 16):  # v1 all
                    steps0[i]()
                for i in (2, 3, 6, 7):  # qk1 jt=1, jt=3 (pair1)
                    steps0[i]()

                for a in range(NCH):
                    ch = slice(a * TCH, (a + 1) * TCH)
                    nblk = 4 * a + 4  # causal: k-blocks 0..4a+3
                    if a + 1 < NCH:
                        urgent.extend(make_qkv_steps(a + 1))
                    for h in range(HPC):
                        pair, lane = h // 2, h % 2
                        py = ps_y.tile(
                            [HD + 1, TCH], F32, tag="py", name=f"py{a}_{h}"
                        )
                        # software-pipeline: AV(j) is enqueued after S(j+3) so
                        # the in-order PE never stalls waiting for exp(j)
                        DEPTH = 4
                        pts = {}

                        def emit_s(j, pair=pair, lane=lane, a=a):
                            r = j - 4 * a
                            # diagonal blocks: cols t_q < 128*r are fully
                            # masked — skip them in S, exp, mask and AV
                            c0 = 128 * r if r > 0 else 0
                            ps = ps_s.tile([P, TCH], F32, tag="ps")
                            nc.tensor.matmul(
                                ps[:, c0:],
                                kts[pair][
                                    64 * lane : 64 * lane + 64,
                                    j * P : (j + 1) * P,
                                ],
                                qts[pair][
                                    64 * lane : 64 * lane + 64,
                                    a * TCH + c0 : (a + 1) * TCH,
                                ],
                                start=True,
                                stop=True,
                            )
                            pt = ptp.tile([P, TCH], BF16)
                            nc.scalar.activation(pt[:, c0:], ps[:, c0:], AF.Exp)
                            if r >= 0:
                                # only the 128-wide staircase needs masking;
                                # Pool (idle otherwise) keeps DVE off the
                                # exp->mask->AV critical path
                                nc.gpsimd.tensor_mul(
                                    pt[:, c0 : c0 + P],
                                    pt[:, c0 : c0 + P],
                                    mask_sb[:],
                                )
                            pts[j] = (pt, c0)

                        def emit_av(j, h=h, py=py, nblk=nblk):
                            pt, c0 = pts.pop(j)
                            nc.tensor.matmul(
                                py[:, c0:],
                                v_sb[:, j, h, :],
                                pt[:, c0:],
                                start=(j == 0),
                                stop=(j == nblk - 1),
                            )

                        for j in range(nblk):
                            emit_s(j)
                            pop_carry()
                            if j >= DEPTH:
                                emit_av(j - DEPTH)
                        for j in range(max(0, nblk - DEPTH), nblk):
                            emit_av(j)
                            pop_carry()
                        # stash unnormalized y^T; the denominator row
                        # (PSUM partition 64) hops to l4 partition h via an
                        # ACT stage copy (exp's act table includes 'copy')
                        # + SBUF->SBUF DMA (DMA sources can't be PSUM, and
                        # engines can't write partitions 1..31)
                        nc.vector.tensor_copy(
                            yT_sb[64 * lane : 64 * lane + 64, pair, ch],
                            py[0:HD, :],
                        )
                        lt = lstp.tile([1, TCH], F32, tag="lst", name=f"lt{a}_{h}")
                        nc.scalar.activation(lt[:], py[HD : HD + 1, :], AF.Copy)
                        nc.sync.dma_start(l4_sb[h : h + 1, a, :], lt[:])
                        if h == 3:
                            lazy.extend(norm_steps(a))
                    # anything of next chunk's QKV not yet popped MUST run
                    # before its attention starts
                    while urgent:
                        urgent.pop(0)()
                    lazy.extend(proj_steps(a))
                # endgame: normalize pair1 of chunk 3 + its projection
                while lazy:
                    lazy.pop(0)()

    nc.compile()
    return nc


def _in_maps(x, w_attn, b_attn, w_proj):
    """Build the 8 per-core input maps (cid = 4*b + g)."""
    import ml_dtypes

    bf16 = ml_dtypes.bfloat16
    i = np.arange(P)
    mask_tri = (i[:, None] <= i[None, :]).astype(bf16)
    emat = np.zeros((4, 2, P), np.float32)
    for pair in range(2):
        emat[2 * pair, pair, 0:64] = 1.0
        emat[2 * pair + 1, pair, 64:P] = 1.0

    wq, wk, wvv = w_attn[:, 0:C], w_attn[:, C : 2 * C], w_attn[:, 2 * C : 3 * C]
    bq, bk, bvv = b_attn[0:C], b_attn[C : 2 * C], b_attn[2 * C : 3 * C]

    maps = []
    for b in range(B):
        xTb = np.ascontiguousarray(x[b].T.astype(bf16))
        for g in range(4):
            s = slice(g * DC, (g + 1) * DC)
            wqk_c = np.ascontiguousarray(
                np.concatenate([wq[:, s], wk[:, s]], axis=1).astype(bf16)
            )
            bqk_c = np.stack(
                [
                    0.125 * bq[s][0:P],
                    0.125 * bq[s][P:DC],
                    bk[s][0:P],
                    bk[s][P:DC],
                ],
                axis=1,
            ).astype(np.float32)
            maps.append(
                {
                    "xT": xTb,
                    "wqk": wqk_c,
                    "wv": np.ascontiguousarray(wvv[:, s].astype(bf16)),
                    "wproj": np.ascontiguousarray(w_proj[s, :].astype(bf16)),
                    "bqk": np.ascontiguousarray(bqk_c),
                    "bv": np.ascontiguousarray(
                        np.concatenate(
                            [
                                bvv[s].reshape(HPC, HD),
                                np.ones((HPC, 1), np.float32),
                            ],
                            axis=1,
                        ).reshape(1, HPC, HD + 1).astype(np.float32)
                    ),
                    "mask_tri": mask_tri,
                    "emat": emat,
                }
            )
    return maps


def run(x, w_attn, b_attn, w_proj, b_proj, trace=False):
    from concourse.bass_utils import run_bass_kernel_spmd

    if "nc" not in _CACHE:
        _CACHE["nc"] = _build()
    nc = _CACHE["nc"]
    maps = _in_maps(
        np.asarray(x), np.asarray(w_attn), np.asarray(b_attn), np.asarray(w_proj)
    )
    r = run_bass_kernel_spmd(nc, maps, core_ids=list(range(8)), trace=trace)
    partials = [r.results[i]["out"] for i in range(8)]
    bp = np.asarray(b_proj, dtype=np.float32)
    y = np.stack(
        [sum(partials[4 * b : 4 * b + 4]) + bp for b in range(B)], axis=0
    ).astype(np.float32)
    return y, r


def kernel(x, w_attn, b_attn, w_proj, b_proj):
    y, _ = run(x, w_attn, b_attn, w_proj, b_proj, trace=False)
    return y
